# revision 48
# baseline (speedup 1.0000x reference)
"""Two-layer GCN encoder on 8 Trainium2 NeuronCores (Bass/Tile).

Strategy (edge-parallel by destination range):
  - Sort edges by dst on the host; core k owns dst range [6400k, 6400(k+1)).
    All edges for a dst land on one core, so segment sums are locally exact.
  - Degree / D^{-1/2} normalization is a pure function of edge_index, so it
    is precomputed on the host along with the edge sort (format conversion);
    all x/W-dependent math runs on device.
  - GCN algebra refactored so all per-edge work is gather + segment-sum;
    relu's positive homogeneity pulls the layer-1 dis scale through:
      zt = dis^2 * relu(S1@W1 + invdis*b1) @ W2.
  - Per-edge segment-sum on the tensor engine with dst as the OUTPUT
    partition dim: acc[128dst, width] += onehot(dst)^T @ msg, so each matmul
    costs ~width cycles (5 for layer 1, 64 for layer 2) instead of 128.
  - Gathers use the GPSIMD dma_gather custom op (fp16 tables with 256B rows).
    int16 index limit is handled by splitting each tile's edge list into
    src<32768 and src>=32768 streams gathered from the table halves.
  - zt is distributed with 4 quarter AllGathers overlapped with the layer-1
    tail, then re-laid into the 256B-stride gather table.
"""
import sys

sys.path.insert(0, "/opt/trn_rl_repo")

import numpy as np

from concourse import bacc, mybir, tile
from concourse import library_config
from concourse.bass import BassGpSimd
from concourse.bass_utils import run_bass_kernel_spmd

P = 128
NCORES = 8
N_NODES = 50000
RANGE = 6400                  # nodes per core (50 tiles of 128)
NT = RANGE // P               # 50 node tiles per core
NG = NCORES * NT              # 400 global node tiles
V = NCORES * RANGE            # 51200 padded table rows
HALF = 32768                  # int16 index split point
F2 = 64                       # zt cols
FX = 5                        # raw x feature count
TBLW = 128                    # table row width (fp16 -> 256B rows)
GT = 1                        # tiles per gather group
PAD_DST = 9999                # one-hot miss value for padded edge slots
NQ = 5                        # zt AllGather chunks
QTS = [16, 14, 10, 6, 4]      # tiles per chunk (sums to 50); decreasing sizes
                              # so the serial collective chain tracks the
                              # linear zt arrival and the last chunk is short

f16 = mybir.dt.float16
f32 = mybir.dt.float32
i16 = mybir.dt.int16

_prog_cache = {}


def build_program(cpt_lo, cpt_hi):
    cpt = cpt_lo + cpt_hi
    C = NT * cpt                      # dst16 columns per core
    NGRP = NT // GT
    CL = NT * cpt_lo * 8              # idx_lo columns (128/16 per chunk)
    CH = NT * cpt_hi * 8

    nc = bacc.Bacc("TRN2", target_bir_lowering=False, debug=False,
                   num_devices=NCORES)

    dst_rel = nc.declare_dram_parameter("dst_rel", [P, C], f16, isOutput=False)
    idx_lo = nc.declare_dram_parameter("idx_lo", [P, CL], i16, isOutput=False)
    idx_hi = nc.declare_dram_parameter("idx_hi", [P, CH], i16, isOutput=False)
    x_pad = nc.declare_dram_parameter("x_pad", [P, NG, FX], f16, isOutput=False)
    x_own = nc.declare_dram_parameter("x_own", [P, NT, FX], f16, isOutput=False)
    w1 = nc.declare_dram_parameter("w1", [FX, 128], f16, isOutput=False)
    b1 = nc.declare_dram_parameter("b1", [1, 128], f16, isOutput=False)
    w2 = nc.declare_dram_parameter("w2", [128, F2], f16, isOutput=False)
    b2 = nc.declare_dram_parameter("b2", [F2], f32, isOutput=False)
    iota_in = nc.declare_dram_parameter("iota_in", [P, P * cpt], f16, isOutput=False)
    ident_in = nc.declare_dram_parameter("ident_in", [P, P], f16, isOutput=False)
    # host-precomputed normalization (pure functions of edge_index)
    dis_g_in = nc.declare_dram_parameter("dis_g", [P, NG], f16, isOutput=False)
    dis_cols_in = nc.declare_dram_parameter("dis_cols", [P, NT], f32, isOutput=False)
    dis2_cols_in = nc.declare_dram_parameter("dis2_cols", [P, NT], f32, isOutput=False)
    invdis_in = nc.declare_dram_parameter("invdis", [1, RANGE], f16, isOutput=False)
    out_ext = nc.declare_dram_parameter("out", [RANGE, F2], f32, isOutput=True)

    xs_tbl = nc.dram_tensor("xs_tbl", [V, TBLW], f16)
    ztown_dram = nc.dram_tensor("ztown_dram", [RANGE, F2], f16)
    ztq_dram = [nc.dram_tensor(f"ztq{q}_dram", [NCORES * QTS[q] * P, F2], f16,
                               addr_space="Shared") for q in range(NQ)]
    ztglob_dram = nc.dram_tensor("ztglob_dram", [V, TBLW], f16)

    rg = [list(range(NCORES))]
    mlp = library_config.mlp

    q_of_tile = []                    # tile t -> quarter index, boundary flag
    acc_t = 0
    bounds = []
    for q in range(NQ):
        acc_t += QTS[q]
        bounds.append(acc_t)

    with tile.TileContext(nc) as tc:
        with (
            tc.tile_pool(name="const", bufs=1) as const,
            tc.tile_pool(name="ohp", bufs=5) as ohp,
            tc.tile_pool(name="msgp", bufs=8) as msgp,
            tc.tile_pool(name="smallp", bufs=10) as smallp,
            tc.tile_pool(name="ps_seg", bufs=2, space="PSUM") as ps_seg,
            tc.tile_pool(name="ps_big", bufs=2, space="PSUM") as ps_big,
            tc.tile_pool(name="ps_aux", bufs=2, space="PSUM") as ps_aux,
            tc.tile_pool(name="ps_tr", bufs=2, space="PSUM") as ps_tr,
        ):
            # ---------- constants / inputs ----------
            # xs-table inputs load first: the first layer-1 gather gates on
            # the full table, so its build pipeline leads everything else.
            nc.gpsimd.load_library(mlp)

            dis_g = const.tile([P, NG], f16)
            nc.sync.dma_start(out=dis_g[:], in_=dis_g_in[:])
            idxlo_sb = const.tile([P, CL], i16)
            nc.sync.dma_start(out=idxlo_sb[:], in_=idx_lo[:])
            idxhi_sb = const.tile([P, CH], i16)
            nc.sync.dma_start(out=idxhi_sb[:], in_=idx_hi[:])

            # xs table: cols 0:8 = dis*x (5 used), rest never read; pipelined
            # load -> scale -> thin store in XG-tile steps
            XG = 25                      # global tiles per build step
            with (
                tc.tile_pool(name="xload", bufs=3) as xload,
                tc.tile_pool(name="xsbuild", bufs=3) as xsbuild,
            ):
                for s in range(NG // XG):
                    xraw = xload.tile([P, XG, FX], f16, tag="xraw")
                    nc.sync.dma_start(out=xraw[:],
                                      in_=x_pad[:, s * XG:(s + 1) * XG, :])
                    xsb = xsbuild.tile([P, XG, 8], f16, tag="xsb")
                    nc.vector.tensor_tensor(
                        out=xsb[:, :, 0:FX],
                        in0=xraw[:],
                        in1=dis_g[:, s * XG:(s + 1) * XG, None].broadcast_to(
                            [P, XG, FX]),
                        op=mybir.AluOpType.mult,
                    )
                    nc.sync.dma_start(
                        out=xs_tbl.ap()[s * XG * P:(s + 1) * XG * P, 0:8].rearrange(
                            "(g p) f -> p g f", p=P),
                        in_=xsb[:],
                    )

            iota16 = const.tile([P, P * cpt], f16)
            nc.sync.dma_start(out=iota16[:], in_=iota_in[:])
            ident = const.tile([P, P], f16)
            nc.sync.dma_start(out=ident[:], in_=ident_in[:])
            dst16 = const.tile([P, C], f16)
            nc.sync.dma_start(out=dst16[:], in_=dst_rel[:])

            w1_sb = const.tile([FX, 128], f16)
            nc.sync.dma_start(out=w1_sb[:], in_=w1[:])
            b1row = const.tile([1, 128], f16)
            nc.sync.dma_start(out=b1row[:], in_=b1[:])
            w2_sb = const.tile([128, F2], f16)
            nc.sync.dma_start(out=w2_sb[:], in_=w2[:])

            ones1_f32 = const.tile([1, P], f32)
            nc.vector.memset(ones1_f32[:], 1.0)
            b2row = const.tile([1, F2], f32)
            nc.sync.dma_start(out=b2row[:], in_=b2[None, :])
            b2psum = ps_aux.tile([P, F2], f32, tag="aux")
            nc.tensor.matmul(out=b2psum[:], lhsT=ones1_f32[:], rhs=b2row[:],
                             start=True, stop=True)
            b2bc = const.tile([P, F2], f32)
            nc.vector.tensor_copy(out=b2bc[:], in_=b2psum[:])

            x_own_sb = const.tile([P, NT, FX], f16)
            nc.sync.dma_start(out=x_own_sb[:], in_=x_own[:])

            dis_cols = const.tile([P, NT], f32)
            nc.sync.dma_start(out=dis_cols[:], in_=dis_cols_in[:])
            dis2_cols = const.tile([P, NT], f32)
            nc.sync.dma_start(out=dis2_cols[:], in_=dis2_cols_in[:])
            invdis_flat = const.tile([1, RANGE], f16)
            nc.sync.dma_start(out=invdis_flat[:], in_=invdis_in[:])

            # own-range xs (f16) for the layer-1 self-loop term
            dis_cols16 = const.tile([P, NT], f16)
            nc.vector.tensor_copy(out=dis_cols16[:], in_=dis_cols[:])
            xs_own = const.tile([P, NT, FX], f16)
            nc.vector.tensor_tensor(
                out=xs_own[:],
                in0=x_own_sb[:],
                in1=dis_cols16[:, :, None].broadcast_to([P, NT, FX]),
                op=mybir.AluOpType.mult,
            )

            ztf32 = const.tile([P, NT, F2], f32)

            def oh_build(oh, t):
                """One-hot for tile t: oh[p, n, c] = (dst[p,c]==n), fp16."""
                q, j = divmod(t, GT)
                lo0 = q * GT * cpt + j * cpt_lo
                hi0 = q * GT * cpt + GT * cpt_lo + j * cpt_hi
                ohv = oh[:].rearrange("p (n c) -> p n c", c=cpt)
                iov = iota16[:].rearrange("p (n c) -> p n c", c=cpt)
                nc.vector.tensor_tensor(
                    out=ohv[:, :, 0:cpt_lo],
                    in0=dst16[:, None, lo0:lo0 + cpt_lo].broadcast_to(
                        [P, P, cpt_lo]),
                    in1=iov[:, :, 0:cpt_lo],
                    op=mybir.AluOpType.is_equal,
                )
                nc.vector.tensor_tensor(
                    out=ohv[:, :, cpt_lo:cpt],
                    in0=dst16[:, None, hi0:hi0 + cpt_hi].broadcast_to(
                        [P, P, cpt_hi]),
                    in1=iov[:, :, cpt_lo:cpt],
                    op=mybir.AluOpType.is_equal,
                )

            def seg_matmuls(acc, oh, msg, j, width, extra):
                """acc[128dst, width] += oh_chunk^T @ msg_chunk over chunks.

                extra: (lhsT, rhs) appended with stop on the last matmul."""
                ohv = oh[:].rearrange("p (n c) -> p n c", c=cpt)
                for i in range(cpt):
                    if i < cpt_lo:
                        mcol = j * cpt_lo + i
                    else:
                        mcol = GT * cpt_lo + j * cpt_hi + (i - cpt_lo)
                    nc.tensor.matmul(
                        out=acc[:], lhsT=ohv[:, :, i],
                        rhs=msg[:, mcol, 0:width],
                        start=(i == 0), stop=False,
                    )
                lhsT_x, rhs_x = extra
                nc.tensor.matmul(out=acc[:], lhsT=lhsT_x, rhs=rhs_x,
                                 start=False, stop=True)

            # ---------- pass 1: layer 1 -> zt ----------
            done_tiles = 0
            for grp in range(NGRP):
                msg = msgp.tile([P, GT * cpt, TBLW], f16, tag="msg")
                nlo = GT * cpt_lo * P
                nhi = GT * cpt_hi * P
                nc.gpsimd.dma_gather(
                    msg[:, 0:GT * cpt_lo, :], xs_tbl[0:HALF, :],
                    idxlo_sb[:, grp * GT * cpt_lo * 8:(grp + 1) * GT * cpt_lo * 8],
                    nlo, nlo, TBLW, single_packet=False,
                )
                nc.gpsimd.dma_gather(
                    msg[:, GT * cpt_lo:GT * cpt, :], xs_tbl[HALF:V, :],
                    idxhi_sb[:, grp * GT * cpt_hi * 8:(grp + 1) * GT * cpt_hi * 8],
                    nhi, nhi, TBLW, single_packet=False,
                )
                for j in range(GT):
                    t = grp * GT + j
                    oh = ohp.tile([P, cpt * P], f16, tag="oh")
                    oh_build(oh, t)
                    s1p = ps_seg.tile([P, FX], f32, tag="seg")
                    seg_matmuls(s1p, oh, msg, j, FX,
                                (ident[:], xs_own[:, t, :]))
                    s1sb = smallp.tile([P, FX], f16, tag="s1sb")
                    nc.scalar.copy(out=s1sb[:], in_=s1p[:])
                    s1tp = ps_tr.tile([FX, P], f16, tag="tr")
                    nc.tensor.transpose(out=s1tp[:], in_=s1sb[:],
                                        identity=ident[:])
                    s1t = smallp.tile([FX, P], f16, tag="s1t")
                    nc.scalar.copy(out=s1t[:], in_=s1tp[:])
                    h1p = ps_big.tile([P, P], f32, tag="h1")
                    nc.tensor.matmul(out=h1p[:], lhsT=w1_sb[:], rhs=s1t[:],
                                     start=True, stop=False)
                    nc.tensor.matmul(out=h1p[:], lhsT=b1row[:],
                                     rhs=invdis_flat[:, t * P:(t + 1) * P],
                                     start=False, stop=True)
                    h1r = smallp.tile([P, P], f16, tag="h1r")
                    nc.scalar.activation(out=h1r[:], in_=h1p[:],
                                         func=mybir.ActivationFunctionType.Relu)
                    ztp = ps_aux.tile([P, F2], f32, tag="aux")
                    nc.tensor.matmul(out=ztp[:], lhsT=h1r[:], rhs=w2_sb[:],
                                     start=True, stop=True)
                    nc.vector.tensor_tensor(
                        out=ztf32[:, t, :], in0=ztp[:],
                        in1=dis2_cols[:, t:t + 1].to_broadcast([P, F2]),
                        op=mybir.AluOpType.mult,
                    )
                    zt16 = smallp.tile([P, F2], f16, tag="zt16")
                    nc.scalar.copy(out=zt16[:], in_=ztf32[:, t, :])
                    nc.sync.dma_start(out=ztown_dram[t * P:(t + 1) * P, :],
                                      in_=zt16[:])
                    done_tiles = t + 1
                    for q in range(NQ):
                        if done_tiles == bounds[q]:
                            lo_t = bounds[q] - QTS[q]
                            nc.gpsimd.collective_compute(
                                "AllGather", mybir.AluOpType.bypass,
                                replica_groups=rg,
                                ins=[ztown_dram[lo_t * P:bounds[q] * P, :]],
                                outs=[ztq_dram[q][:]],
                            )
                            # re-lay quarter into the 256B-stride gather table
                            nc.sync.dma_start(
                                out=ztglob_dram.ap().rearrange(
                                    "(k r) f -> k r f",
                                    k=NCORES)[:, lo_t * P:bounds[q] * P, 0:F2],
                                in_=ztq_dram[q].ap().rearrange(
                                    "(k r) f -> k r f", k=NCORES),
                            )

            # ---------- pass 2: layer 2 -> output ----------
            for grp in range(NGRP):
                msg = msgp.tile([P, GT * cpt, TBLW], f16, tag="msg")
                nlo = GT * cpt_lo * P
                nhi = GT * cpt_hi * P
                nc.gpsimd.dma_gather(
                    msg[:, 0:GT * cpt_lo, :], ztglob_dram[0:HALF, :],
                    idxlo_sb[:, grp * GT * cpt_lo * 8:(grp + 1) * GT * cpt_lo * 8],
                    nlo, nlo, TBLW, single_packet=False,
                )
                nc.gpsimd.dma_gather(
                    msg[:, GT * cpt_lo:GT * cpt, :], ztglob_dram[HALF:V, :],
                    idxhi_sb[:, grp * GT * cpt_hi * 8:(grp + 1) * GT * cpt_hi * 8],
                    nhi, nhi, TBLW, single_packet=False,
                )
                for j in range(GT):
                    t = grp * GT + j
                    oh = ohp.tile([P, cpt * P], f16, tag="oh")
                    oh_build(oh, t)
                    g2p = ps_seg.tile([P, F2], f32, tag="seg")
                    ohv = oh[:].rearrange("p (n c) -> p n c", c=cpt)
                    for i in range(cpt):
                        if i < cpt_lo:
                            mcol = j * cpt_lo + i
                        else:
                            mcol = GT * cpt_lo + j * cpt_hi + (i - cpt_lo)
                        nc.tensor.matmul(
                            out=g2p[:], lhsT=ohv[:, :, i],
                            rhs=msg[:, mcol, 0:F2],
                            start=(i == 0), stop=(i == cpt - 1),
                        )
                    sum_sb = smallp.tile([P, F2], f32, tag="sum")
                    nc.vector.tensor_add(out=sum_sb[:], in0=g2p[:],
                                         in1=ztf32[:, t, :])
                    out_sb = smallp.tile([P, F2], f32, tag="outt")
                    nc.vector.scalar_tensor_tensor(
                        out=out_sb[:], in0=sum_sb[:],
                        scalar=dis_cols[:, t:t + 1], in1=b2bc[:],
                        op0=mybir.AluOpType.mult, op1=mybir.AluOpType.add,
                    )
                    nc.sync.dma_start(out=out_ext[t * P:(t + 1) * P, :],
                                      in_=out_sb[:])

    nc.compile()
    return nc


def _serpentine_group(nodes, tiles, lo_k, hi_k, assign_tile):
    """Serpentine-stripe `nodes` (sorted by lo desc) across `tiles`, then
    repair swaps toward per-tile chunk caps. Mutates assign_tile[nodes]."""
    ntl = len(tiles)
    rounds = len(nodes) // ntl
    order = nodes[np.argsort(-(lo_k[nodes] * 10000 + hi_k[nodes]),
                             kind="stable")]
    for r in range(rounds):
        row = order[r * ntl:(r + 1) * ntl]
        tl = tiles if r % 2 == 0 else tiles[::-1]
        assign_tile[row] = tl

    t_lo = {t: int(lo_k[order[assign_tile[order] == t]].sum()) for t in tiles}
    t_hi = {t: int(hi_k[order[assign_tile[order] == t]].sum()) for t in tiles}

    def chunk_cap(total):
        return (total // (ntl * P) + 1) * P

    cap_lo = chunk_cap(sum(t_lo.values()))
    cap_hi_big = chunk_cap(sum(t_hi.values())) + P
    cap_hi_try = chunk_cap(sum(t_hi.values()))

    def do_swaps(key_k, other_k, t_key, t_other, cap_key, cap_other, iters):
        for _ in range(iters):
            w = max(tiles, key=lambda t: t_key[t])
            if t_key[w] <= cap_key:
                return True
            need = t_key[w] - cap_key
            v = min(tiles, key=lambda t: t_key[t])
            headroom = cap_key - t_key[v]
            in_w = order[assign_tile[order] == w]
            in_v = order[assign_tile[order] == v]
            aa = in_w[np.argsort(-key_k[in_w])][:24]
            bb = in_v[np.argsort(key_k[in_v])][:24]
            best = None
            for a in aa:
                for b in bb:
                    gain = key_k[a] - key_k[b]
                    if gain <= 0 or gain > headroom:
                        continue
                    doth = other_k[a] - other_k[b]
                    if t_other[v] + doth > cap_other:
                        continue
                    if t_other[w] - doth > cap_other:
                        continue
                    sc = min(gain, need) * 4 - abs(doth)
                    if best is None or sc > best[0]:
                        best = (sc, a, b)
            if best is None:
                return False
            _, a, b = best
            assign_tile[a], assign_tile[b] = v, w
            t_key[w] += key_k[b] - key_k[a]
            t_key[v] += key_k[a] - key_k[b]
            t_other[w] += other_k[b] - other_k[a]
            t_other[v] += other_k[a] - other_k[b]
        return False

    do_swaps(lo_k, hi_k, t_lo, t_hi, cap_lo, cap_hi_big, 300)
    saved = (assign_tile[order].copy(), dict(t_lo), dict(t_hi))
    ok = do_swaps(hi_k, lo_k, t_hi, t_lo, cap_hi_try, cap_lo, 300)
    if not ok or max(t_lo.values()) > cap_lo:
        assign_tile[order] = saved[0]


def _balance_tiles(src, dst):
    """Permute each core's 6400 dst nodes across its 50 tiles so per-tile
    lo/hi edge counts are balanced (shrinks the padded chunk count cpt).

    lo/hi refers to the gather table half (slot < 32768, int16 index limit).
    Core HALF//RANGE straddles the boundary: its lo window (768 slots) is
    given to its lowest out-degree nodes, shifting edges from the tighter lo
    stream into hi globally.

    Returns slot_of[v] (global node id -> table/tile slot) and perm (slot ->
    global node id)."""
    KS = HALF // RANGE                   # straddling core (5)
    NWIN = HALF - KS * RANGE             # its lo-window slot count (768)
    outdeg = np.bincount(src, minlength=V)
    c5 = np.arange(KS * RANGE, (KS + 1) * RANGE)
    w_order = np.argsort(outdeg[c5], kind="stable")
    win_nodes = c5[w_order[:NWIN]]
    src_is_lo = src < KS * RANGE
    is_win = np.zeros(V, bool)
    is_win[win_nodes] = True
    src_is_lo |= is_win[src]

    lo_cnt = np.bincount(dst[src_is_lo], minlength=V).astype(np.int64)
    hi_cnt = np.bincount(dst[~src_is_lo], minlength=V).astype(np.int64)

    slot_of = np.empty(V, np.int64)
    perm = np.empty(V, np.int64)
    for k in range(NCORES):
        nodes = np.arange(k * RANGE, (k + 1) * RANGE)
        assign_tile = np.empty(V, np.int64)
        if k == KS:
            rest = nodes[~is_win[nodes]]
            _serpentine_group(win_nodes, np.arange(NWIN // P), lo_cnt, hi_cnt,
                              assign_tile)
            _serpentine_group(rest, np.arange(NWIN // P, NT), lo_cnt, hi_cnt,
                              assign_tile)
        else:
            _serpentine_group(nodes, np.arange(NT), lo_cnt, hi_cnt,
                              assign_tile)
        t_n = np.zeros(NT, np.int64)
        for n in nodes:
            t = assign_tile[n]
            s = k * RANGE + t * P + t_n[t]
            slot_of[n] = s
            perm[s] = n
            t_n[t] += 1
    return slot_of, perm


def _prepare_shards(src, dst):
    """Group edges by dst tile, split into lo/hi src streams, pad to uniform
    chunk counts, and emit device arrays in the group-major slot layout.
    src/dst are SLOT ids (post tile-balancing permutation)."""
    E = src.shape[0]
    tile_g = dst >> 7

    hi_mask0 = src >= HALF
    # order: by tile, lo stream first, stable
    sub_order = np.lexsort((np.arange(E), hi_mask0.astype(np.int8), tile_g))
    ssrc = src[sub_order]
    stile = tile_g[sub_order]
    sdst = dst[sub_order]
    hi_mask = ssrc >= HALF

    lo_counts = np.bincount(stile[~hi_mask], minlength=NG)
    hi_counts = np.bincount(stile[hi_mask], minlength=NG)
    cpt_lo = max(1, int(np.ceil(lo_counts.max() / P)))
    cpt_hi = max(1, int(np.ceil(hi_counts.max() / P)))
    cap_lo, cap_hi = cpt_lo * P, cpt_hi * P

    tile_starts = np.zeros(NG + 1, np.int64)
    np.cumsum(lo_counts + hi_counts, out=tile_starts[1:])
    pos_in_tile = np.arange(E, dtype=np.int64) - tile_starts[stile]
    within = np.where(hi_mask, pos_in_tile - lo_counts[stile], pos_in_tile)

    src_lo = np.zeros((NG, cap_lo), np.int16)          # pad -> row 0
    dst_lo = np.full((NG, cap_lo), PAD_DST, np.int32)
    src_hi = np.zeros((NG, cap_hi), np.int16)
    dst_hi = np.full((NG, cap_hi), PAD_DST, np.int32)
    lo_sel = ~hi_mask
    src_lo[stile[lo_sel], within[lo_sel]] = ssrc[lo_sel].astype(np.int16)
    dst_lo[stile[lo_sel], within[lo_sel]] = sdst[lo_sel] & 127
    src_hi[stile[hi_mask], within[hi_mask]] = (ssrc[hi_mask] - HALF).astype(
        np.int16)
    dst_hi[stile[hi_mask], within[hi_mask]] = sdst[hi_mask] & 127

    cpt = cpt_lo + cpt_hi
    NGRP = NT // GT
    dst16_dev, idxlo_dev, idxhi_dev = [], [], []

    def wrap(a):
        lin = a.reshape(-1)
        w = lin.reshape(-1, 16).T          # idx j -> [j%16, j//16]
        return np.ascontiguousarray(np.tile(w, (8, 1)))

    for k in range(NCORES):
        tl = slice(k * NT, (k + 1) * NT)
        klo_src = src_lo[tl].reshape(NGRP, GT, cpt_lo, P)
        klo_dst = dst_lo[tl].reshape(NGRP, GT, cpt_lo, P)
        khi_src = src_hi[tl].reshape(NGRP, GT, cpt_hi, P)
        khi_dst = dst_hi[tl].reshape(NGRP, GT, cpt_hi, P)

        # dst16 [P, NGRP*(GT*cpt)]; per-group cols [lo(t0) lo(t1) hi(t0) hi(t1)]
        dcols = np.concatenate(
            [klo_dst.reshape(NGRP, GT * cpt_lo, P),
             khi_dst.reshape(NGRP, GT * cpt_hi, P)], axis=1)
        d16 = dcols.transpose(2, 0, 1).reshape(P, NT * cpt).astype(np.float16)
        dst16_dev.append(np.ascontiguousarray(d16))

        idxlo_dev.append(wrap(klo_src))
        idxhi_dev.append(wrap(khi_src))

    return cpt_lo, cpt_hi, dst16_dev, idxlo_dev, idxhi_dev


def make_inputs(x, edge_index, W1, b1, W2, b2):
    x = np.asarray(x, np.float32)
    ei = np.asarray(edge_index)
    src = ei[0].astype(np.int64)
    dst = ei[1].astype(np.int64)

    slot_of, perm = _balance_tiles(src, dst)
    cpt_lo, cpt_hi, dst16_dev, idxlo_dev, idxhi_dev = _prepare_shards(
        slot_of[src], slot_of[dst])

    # normalization tables: deg includes the self-loop; pad rows get dis=1
    deg = np.bincount(dst, minlength=N_NODES).astype(np.float32) + 1.0
    dis = 1.0 / np.sqrt(deg)
    dis_node = np.ones(V, np.float32)
    dis_node[:N_NODES] = dis
    dis_pad = dis_node[perm]             # slot-ordered

    x_node = np.zeros((V, FX), np.float32)
    x_node[:N_NODES] = x
    x_padded = x_node[perm]              # slot-ordered
    x_dev = np.ascontiguousarray(x_padded.reshape(NG, P, FX).transpose(1, 0, 2))
    dis_g_dev = np.ascontiguousarray(dis_pad.reshape(NG, P).T)
    cpt = cpt_lo + cpt_hi
    iota = np.tile(np.repeat(np.arange(P, dtype=np.float16), cpt)[None, :],
                   (P, 1))
    ident = np.eye(P, dtype=np.float16)

    in_maps = []
    for k in range(NCORES):
        dis_own = dis_pad[k * RANGE:(k + 1) * RANGE].reshape(NT, P).T
        x_own_k = np.ascontiguousarray(
            x_dev[:, k * NT:(k + 1) * NT, :]).astype(np.float16)
        in_maps.append({
            "dst_rel": dst16_dev[k],
            "idx_lo": idxlo_dev[k],
            "idx_hi": idxhi_dev[k],
            "x_pad": x_dev.astype(np.float16), "x_own": x_own_k,
            "w1": np.asarray(W1).astype(np.float16),
            "b1": np.asarray(b1).astype(np.float16)[None, :],
            "w2": np.asarray(W2).astype(np.float16),
            "b2": np.asarray(b2, np.float32),
            "iota_in": iota, "ident_in": ident,
            "dis_g": dis_g_dev.astype(np.float16),
            "dis_cols": np.ascontiguousarray(dis_own.astype(np.float32)),
            "dis2_cols": np.ascontiguousarray((dis_own ** 2).astype(np.float32)),
            "invdis": np.ascontiguousarray(
                (1.0 / dis_pad[k * RANGE:(k + 1) * RANGE])[None, :].astype(
                    np.float16)),
        })
    return (cpt_lo, cpt_hi), in_maps, perm


def kernel(x, edge_index, W1, b1, W2, b2):
    key, in_maps, perm = make_inputs(x, edge_index, W1, b1, W2, b2)
    if key not in _prog_cache:
        _prog_cache[key] = build_program(*key)
    nc = _prog_cache[key]
    res = run_bass_kernel_spmd(nc, in_maps, list(range(NCORES)))
    res_slots = np.concatenate(
        [res.results[k]["out"] for k in range(NCORES)], axis=0)
    out = np.empty_like(res_slots)
    out[perm] = res_slots                 # slot order -> node order
    return out[:N_NODES]
